# revision 1
# baseline (speedup 1.0000x reference)
"""MeshRefineNet GNN on 8 TRN2 cores — dma_gather-based aggregation (v2).

v1 gathered neighbor rows with per-tile indirect DMAs (128 descriptors each,
~1us fixed SWDGE cost -> 10ms on GPSIMD).  v2 stages each batch's half-edge
rows with a few large dma_gather calls instead (no per-128-row fixed cost,
~8.5ns/descriptor Q7 generation), which
requires int16 indices: the replica is addressed through 10 windows of 32K
rows, and each batch's staged rows are sorted by (window, group, source row).
Tiles of 128 staged rows span at most 2 destination groups; a bf16 rank table
plus one is_equal builds the 0/1 scatter matrices S (width 256 = 2 group
blocks), and TensorE accumulates X_t^T @ S_t per group in PSUM.  The uniform
(max-over-cores) segment lengths keep the compiled program SPMD-identical;
per-core tables carry the indices/ranks.  Transforms, residual, and the
per-layer AllGather are unchanged from v1.
"""
import sys

if "/opt/trn_rl_repo" not in sys.path:
    sys.path.insert(0, "/opt/trn_rl_repo")

import numpy as np

P = 128
SUBW = 32
GB = 16            # groups per batch
WIN = 32768        # dma_gather int16 window (rows)
MAX_RUN = 8192     # split longer runs (descriptor-ring safety)


class Cfg:
    def __init__(self, N, E, NC=8, GROUPS=313):
        self.N, self.E, self.NC, self.GROUPS = N, E, NC, GROUPS
        self.SLOTS = GROUPS * 128
        self.NREP = NC * self.SLOTS
        self.NWIN = -(-self.NREP // WIN)
        self.NB = -(-GROUPS // GB)
        assert self.NREP >= N


CFG_FULL = Cfg(N=320000, E=960000, NC=8, GROUPS=313)


# ----------------------------------------------------------------- host prep
def build_tables(edges, cfg):
    N, NC, GROUPS = cfg.N, cfg.NC, cfg.GROUPS
    SLOTS, NB, NWIN = cfg.SLOTS, cfg.NB, cfg.NWIN

    src0 = edges[:, 0].astype(np.int64)
    dst0 = edges[:, 1].astype(np.int64)
    tgt = np.concatenate([src0, dst0])
    src = np.concatenate([dst0, src0])
    deg = np.bincount(tgt, minlength=N)

    # balanced vertex->slot assignment (degree snake over 32-row bins)
    NBINS = NC * GROUPS * 4
    n_items = NBINS * SUBW
    deg_ext = np.concatenate([deg, np.zeros(n_items - N, np.int64)])
    order = np.argsort(-deg_ext, kind="stable")
    grid = order.reshape(SUBW, NBINS).copy()
    for r in range(1, SUBW, 2):
        grid[r] = grid[r, ::-1]
    item_ids = grid.ravel()
    rr, bb = np.divmod(np.arange(SUBW * NBINS), NBINS)
    core_of = np.empty(n_items, np.int64)
    slot_of = np.empty(n_items, np.int64)
    core_of[item_ids] = bb // (GROUPS * 4)
    g_item = (bb % (GROUPS * 4)) // 4
    slot_of[item_ids] = g_item * 128 + (bb % 4) * SUBW + rr
    H0 = min(160 * 128, SLOTS)          # first-10-batches rows
    half = (slot_of >= H0).astype(np.int64)
    R = np.where(half == 0,
                 core_of * H0 + slot_of,
                 NC * H0 + core_of * (SLOTS - H0) + (slot_of - H0))

    vert_at = np.full((NC, SLOTS), -1, np.int64)
    vert_at[core_of[:N], slot_of[:N]] = np.arange(N)

    c_t = core_of[tgt]
    s_t = slot_of[tgt]
    g_t = s_t // 128
    b_t = g_t // GB
    Rs = R[src]
    w_t = Rs // WIN

    # per (core, batch, window, group) counts -> uniform max lengths
    key = ((c_t * NB + b_t) * NWIN + w_t) * GROUPS + g_t
    cnt = np.bincount(key, minlength=NC * NB * NWIN * GROUPS) \
        .reshape(NC, NB, NWIN, GROUPS)
    Lmax = cnt.max(axis=0)          # [NB, NWIN, GROUPS]

    # ---- uniform walk: tiles / runs / mm schedule (core-independent) ----
    structure = []
    batchoff = []
    total = 0
    segstart = {}
    blockmap = {}                   # (tile_global, g) -> block index
    for b in range(NB):
        glo = b * GB
        ghi = min((b + 1) * GB, GROUPS)
        pos = 0
        runs = []
        tile_groups = []            # batch-local tile -> [g0, g1?]
        mms = []                    # (tile_local, block, g)
        for w in range(NWIN):
            run_start = pos
            for g in range(glo, ghi):
                L = int(Lmax[b, w, g])
                if L == 0:
                    continue
                t = pos // 128
                if pos % 128 != 0:
                    tg = tile_groups[t]
                    if g not in tg and len(tg) == 3:
                        pos = (t + 1) * 128
                segstart[(b, w, g)] = pos
                while L > 0:
                    t = pos // 128
                    while t >= len(tile_groups):
                        tile_groups.append([])
                    tg = tile_groups[t]
                    if g not in tg:
                        assert len(tg) < 3
                        tg.append(g)
                        mms.append((t, tg.index(g), g))
                        blockmap[(total // 128 + t, g)] = tg.index(g)
                    room = (t + 1) * 128 - pos
                    step = min(room, L)
                    pos += step
                    L -= step
            if pos % 128:
                pos = (pos // 128 + 1) * 128
            if pos > run_start:
                runs.append((w, run_start, pos - run_start))
        T_b = pos // 128
        # group-major MM lists (one PSUM bank session per group)
        gmms = {g: [] for g in range(glo, ghi)}
        for (t, blk, g) in mms:
            gmms[g].append((t, blk))
        for g in range(glo, ghi):
            assert gmms[g], f"group {g} empty in batch {b}"
        # compacted S2 slots for tiles holding a 3rd group
        s2loc = {}
        for t, tg in enumerate(tile_groups):
            if len(tg) == 3:
                s2loc[t] = len(s2loc)
        structure.append(dict(T=T_b, runs=runs, gmms=gmms,
                              glo=glo, ng=ghi - glo, s2loc=s2loc,
                              s2off=sum(len(st["s2loc"])
                                        for st in structure)))
        batchoff.append(total)
        total += T_b * 128

    T_total = total // 128
    T2_total = sum(len(st["s2loc"]) for st in structure)

    # ---- per-core tables: staged position of every half-edge ----
    # order within a segment: by (slot, source row)
    seg_of = ((c_t * NB + b_t) * NWIN + w_t) * GROUPS + g_t
    eorder = np.lexsort((Rs, s_t, seg_of))
    so = seg_of[eorder]
    first_in_seg = np.searchsorted(so, np.arange(NC * NB * NWIN * GROUPS))
    cum = np.arange(so.size) - first_in_seg[so]

    segstart_arr = np.zeros((NB, NWIN, GROUPS), np.int64)
    for (b, w, g), v in segstart.items():
        segstart_arr[b, w, g] = v
    boff_arr = np.asarray(batchoff, np.int64)

    et = eorder
    pos_local = segstart_arr[b_t[et], w_t[et], g_t[et]] + cum
    pos_global = boff_arr[b_t[et]] + pos_local
    tile_g = pos_global // 128
    part_g = pos_global % 128

    blk_of = np.zeros(et.size, np.int64)
    bm_keys = np.array([k[0] * GROUPS + k[1] for k in blockmap], np.int64)
    bm_vals = np.array(list(blockmap.values()), np.int64)
    bm = np.zeros(T_total * GROUPS, np.int64)
    bm[bm_keys] = bm_vals
    blk_of = bm[tile_g * GROUPS + g_t[et]]

    idx_lin = np.zeros((NC, total), np.int16)
    rank_lin = np.full((NC, total), 512.0, np.float32)
    rank2_lin = np.full((NC, total), 512.0, np.float32)
    cc = c_t[et]
    idx_lin[cc, pos_global] = (Rs[et] % WIN).astype(np.int16)
    r1v = np.where(blk_of < 2, s_t[et] % 128 + 128 * blk_of, 512)
    r2v = np.where(blk_of == 2, s_t[et] % 128, 512)
    rank_lin[cc, pos_global] = r1v.astype(np.float32)
    rank2_lin[cc, pos_global] = r2v.astype(np.float32)

    # wrapped idx layout per run: local i -> (partition i%16 (+16k), col i//16)
    idx16 = np.zeros((NC, 128, total // 16), np.int16)
    for b in range(NB):
        for (w, start, L) in structure[b]["runs"]:
            s0 = batchoff[b] + start
            blkv = idx_lin[:, s0:s0 + L].reshape(NC, L // 16, 16) \
                .transpose(0, 2, 1)                       # [NC, 16, L/16]
            idx16[:, :, s0 // 16:(s0 + L) // 16] = np.tile(blkv, (1, 8, 1))

    rank_tbl = rank_lin.reshape(NC, T_total, 128).transpose(0, 2, 1).copy()
    rank2_full = rank2_lin.reshape(NC, T_total, 128).transpose(0, 2, 1)
    rank2_tbl = np.full((NC, 128, max(T2_total, 1)), 512.0, np.float32)
    for b, st in enumerate(structure):
        bt0 = batchoff[b] // 128
        for t, p2 in st["s2loc"].items():
            rank2_tbl[:, :, st["s2off"] + p2] = rank2_full[:, :, bt0 + t]

    return dict(idx16=idx16, rank=rank_tbl, rank2=rank2_tbl,
                vert_at=vert_at,
                structure=structure, batchoff=batchoff, T_total=T_total,
                T2_total=T2_total,
                T_max=max(s["T"] for s in structure),
                T2_max=max(len(s["s2loc"]) for s in structure))


def permute_rows(x, vert_at, cfg):
    out = np.zeros((cfg.NC, cfg.SLOTS, x.shape[1]), x.dtype)
    m = vert_at >= 0
    out[m] = x[vert_at[m]]
    return out


# ------------------------------------------------------------- device build
def build_nc(cfg, tables):
    import concourse.bacc as bacc
    import concourse.tile as tile
    import concourse.mybir as mybir
    from concourse import library_config

    BF = mybir.dt.bfloat16
    F32 = mybir.dt.float32
    I16 = mybir.dt.int16
    RELU = mybir.ActivationFunctionType.Relu
    WCOLS = 6 * P + 6

    NC_, GROUPS, SLOTS, NREP = cfg.NC, cfg.GROUPS, cfg.SLOTS, cfg.NREP
    structure = tables["structure"]
    batchoff = tables["batchoff"]
    T_total = tables["T_total"]
    T_max = tables["T_max"]
    HALF = -(-T_max // 2)

    nc = bacc.Bacc(None, target_bir_lowering=False, debug=False)
    xrep0 = nc.declare_dram_parameter("xrep0", [NREP, P], BF, isOutput=False)
    x0sh = nc.declare_dram_parameter("x0sh", [SLOTS, P], BF, isOutput=False)
    idx16 = nc.declare_dram_parameter("idx16", [P, T_total * 8], I16,
                                      isOutput=False)
    rankt = nc.declare_dram_parameter("rankt", [P, T_total], BF,
                                      isOutput=False)
    T2_total = max(tables["T2_total"], 1)
    T2_max = max(tables["T2_max"], 1)
    rankt2 = nc.declare_dram_parameter("rankt2", [P, T2_total], BF,
                                       isOutput=False)
    wts = nc.declare_dram_parameter("wts", [P, WCOLS], BF, isOutput=False)
    iot = nc.declare_dram_parameter("iot", [P, 256], BF, isOutput=False)
    out = nc.declare_dram_parameter("out", [SLOTS, 3], F32, isOutput=True)
    import os as _os
    DBG = bool(int(_os.environ.get("GNN_DBG", "0")))
    if DBG:
        T0 = tables["structure"][0]["T"]
        dbg_stg = nc.declare_dram_parameter("dbg_stg", [P, T0 * P], BF,
                                            isOutput=True)
        dbg_s = nc.declare_dram_parameter("dbg_s", [P, T0 * 256], BF,
                                          isOutput=True)


    xsh = {l: nc.dram_tensor(f"xsh{l}", [SLOTS, P], BF) for l in (1, 2, 3)}
    xrep = {l: nc.dram_tensor(f"xrep{l}", [NREP, P], BF, addr_space="Shared")
            for l in (1, 2, 3)}

    with tile.TileContext(nc) as tc:
        with (
            tc.tile_pool(name="res", bufs=1) as res,
            tc.tile_pool(name="idx", bufs=2) as idx_p,
            tc.tile_pool(name="stg", bufs=2) as stg_p,
            tc.tile_pool(name="s", bufs=2) as s_p,
            tc.tile_pool(name="xt", bufs=2) as xt_p,
            tc.tile_pool(name="x0t", bufs=2) as x0t_p,
            tc.tile_pool(name="xat", bufs=4) as xat_p,
            tc.tile_pool(name="ob", bufs=2) as ob_p,
            tc.tile_pool(name="psA", bufs=6, space="PSUM") as psA,
            tc.tile_pool(name="psB", bufs=2, space="PSUM") as psB,
        ):
            nc.gpsimd.load_library(library_config.attnmlp)
            rank_sb = res.tile([P, T_total], BF)
            nc.sync.dma_start(out=rank_sb[:], in_=rankt.ap())
            rank2_sb = res.tile([P, T2_total], BF)
            nc.sync.dma_start(out=rank2_sb[:], in_=rankt2.ap())
            iota_sb = res.tile([P, HALF * 256], BF)
            nc.sync.dma_start(
                out=iota_sb[:].rearrange("p (t c) -> p t c", c=256),
                in_=iot.ap().rearrange("p (o c) -> p o c", o=1)
                    .to_broadcast([P, HALF, 256]))
            wts_sb = res.tile([P, WCOLS], BF)
            nc.sync.dma_start(out=wts_sb[:], in_=wts.ap())

            for layer in range(4):
                rep_ap = xrep0.ap() if layer == 0 else xrep[layer].ap()
                sh_ap = x0sh.ap() if layer == 0 else xsh[layer].ap()
                if layer < 3:
                    w1 = wts_sb[:, layer * 256: layer * 256 + P]
                    w0 = wts_sb[:, layer * 256 + P: layer * 256 + 2 * P]
                else:
                    w1 = wts_sb[:, 768:771]
                    w0 = wts_sb[:, 771:774]

                H0 = min(160 * 128, SLOTS)
                for b, st in enumerate(structure):
                    T_b, ng, glo = st["T"], st["ng"], st["glo"]
                    boff = batchoff[b]
                    bt0 = boff // 128

                    idxb = idx_p.tile([P, T_max * 8], I16, tag="idx")
                    nc.sync.dma_start(
                        out=idxb[:, 0:T_b * 8],
                        in_=idx16.ap()[:, boff // 16:(boff + T_b * 128) // 16])
                    stg = stg_p.tile([P, T_max, P], BF, tag="stg")
                    for (w, start, L) in st["runs"]:
                        w_hi = min((w + 1) * WIN, NREP)
                        o = 0
                        while o < L:
                            Lc = min(MAX_RUN, L - o)
                            nc.gpsimd.dma_gather(
                                out_ap=stg[:, (start + o) // 128:
                                           (start + o + Lc) // 128, :],
                                in_ap=rep_ap[w * WIN:w_hi, :],
                                idxs_ap=idxb[:, (start + o) // 16:
                                             (start + o + Lc) // 16],
                                num_idxs=Lc,
                                num_idxs_reg=Lc,
                                elem_size=P,
                                single_packet=False,
                            )
                            o += Lc

                    s_tiles = []
                    for hb in range(2):
                        t0 = hb * HALF
                        t1 = min(T_b, (hb + 1) * HALF)
                        if t0 >= t1:
                            s_tiles.append(None)
                            continue
                        S = s_p.tile([P, HALF * 256], BF, tag="s")
                        nc.vector.tensor_tensor(
                            out=S[:, 0:(t1 - t0) * 256]
                                .rearrange("p (t c) -> p t c", c=256),
                            in0=rank_sb[:, bt0 + t0:bt0 + t1]
                                .rearrange("p (t o) -> p t o", o=1)
                                .to_broadcast([P, t1 - t0, 256]),
                            in1=iota_sb[:, 0:(t1 - t0) * 256]
                                .rearrange("p (t c) -> p t c", c=256),
                            op=mybir.AluOpType.is_equal,
                        )
                        s_tiles.append((S, t0))

                    T2_b = len(st["s2loc"])
                    s2l = st["s2loc"]
                    if T2_b:
                        S2b = s_p.tile([P, T2_max * P], BF, tag="s2")
                        nc.vector.tensor_tensor(
                            out=S2b[:, 0:T2_b * P]
                                .rearrange("p (t c) -> p t c", c=P),
                            in0=rank2_sb[:, st["s2off"]:st["s2off"] + T2_b]
                                .rearrange("p (t o) -> p t o", o=1)
                                .to_broadcast([P, T2_b, P]),
                            in1=iota_sb[:, 0:T2_b * 256]
                                .rearrange("p (t c) -> p t c", c=256)
                                [:, :, 0:P],
                            op=mybir.AluOpType.is_equal,
                        )

                    if DBG and layer == 0 and b == 0:
                        nc.sync.dma_start(
                            out=dbg_stg.ap(),
                            in_=stg[:, 0:T_b, :].rearrange("p a b -> p (a b)"))
                        for hb in range(2):
                            if s_tiles[hb] is None:
                                continue
                            S, t0 = s_tiles[hb]
                            t1 = min(T_b, t0 + HALF)
                            nc.sync.dma_start(
                                out=dbg_s.ap()[:, t0 * 256:t1 * 256],
                                in_=S[:, 0:(t1 - t0) * 256])

                    # ---- per-group aggregation + transforms ----
                    g0 = glo
                    xt = xt_p.tile([P, GB * P], BF, tag="xt")
                    nc.sync.dma_start(out=xt[:, 0:ng * P],
                                      in_=sh_ap[g0 * P:(g0 + ng) * P, :],
                                      transpose=True)
                    if layer == 2:
                        x0v = x0t_p.tile([P, GB, P], BF, tag="x0v")
                        nc.sync.dma_start(
                            out=x0v[:, 0:ng, :],
                            in_=x0sh.ap()[g0 * P:(g0 + ng) * P, :]
                                .rearrange("(g p) c -> p g c", p=P))
                    if layer < 3:
                        obuf = ob_p.tile([P, GB, P], BF, tag="ob")
                    else:
                        obuf = ob_p.tile([P, GB, 3], F32, tag="ob3")

                    for gl in range(ng):
                        glist = st["gmms"][glo + gl]
                        pg = psA.tile([P, P], F32, tag="pg")
                        for i, (t, blk) in enumerate(glist):
                            if blk < 2:
                                S, t0 = s_tiles[0] if t < HALF else s_tiles[1]
                                rhs = S[:, (t - t0) * 256 + blk * 128:
                                        (t - t0) * 256 + blk * 128 + 128]
                            else:
                                rhs = S2b[:, s2l[t] * P:(s2l[t] + 1) * P]
                            nc.tensor.matmul(
                                pg[:],
                                lhsT=stg[:, t, :],
                                rhs=rhs,
                                start=(i == 0), stop=(i == len(glist) - 1),
                            )
                        xat = xat_p.tile([P, P], BF)
                        nc.vector.tensor_copy(out=xat[:], in_=pg[:])
                        if layer < 3:
                            ps2 = psB.tile([P, P], F32, tag="ps2")
                            nc.tensor.matmul(ps2[:], lhsT=xat[:], rhs=w1,
                                             start=True, stop=False)
                            nc.tensor.matmul(ps2[:],
                                             lhsT=xt[:, gl * P:(gl + 1) * P],
                                             rhs=w0, start=False, stop=True)
                            if layer == 2:
                                rl = xat_p.tile([P, P], BF, tag="rl")
                                nc.scalar.activation(out=rl[:], in_=ps2[:],
                                                     func=RELU)
                                nc.vector.tensor_add(out=obuf[:, gl, :],
                                                     in0=rl[:],
                                                     in1=x0v[:, gl, :])
                            else:
                                nc.scalar.activation(out=obuf[:, gl, :],
                                                     in_=ps2[:], func=RELU)
                        else:
                            ps2 = psB.tile([P, P], F32, tag="ps2")
                            nc.tensor.matmul(ps2[:, 0:3], lhsT=xat[:], rhs=w1,
                                             start=True, stop=False)
                            nc.tensor.matmul(ps2[:, 0:3],
                                             lhsT=xt[:, gl * P:(gl + 1) * P],
                                             rhs=w0, start=False, stop=True)
                            nc.vector.tensor_copy(out=obuf[:, gl, :],
                                                  in_=ps2[:, 0:3])

                    if layer < 3:
                        dst = xsh[layer + 1].ap()[g0 * P:(g0 + ng) * P, :] \
                            .rearrange("(g p) c -> p g c", p=P)
                    else:
                        dst = out.ap()[g0 * P:(g0 + ng) * P, :] \
                            .rearrange("(g p) c -> p g c", p=P)
                    nc.sync.dma_start(out=dst, in_=obuf[:, 0:ng, :])

                    if layer < 3 and b == 9:
                        nc.gpsimd.collective_compute(
                            "AllGather", mybir.AluOpType.bypass,
                            replica_groups=[list(range(NC_))],
                            ins=[xsh[layer + 1].ap()[0:H0, :].opt()],
                            outs=[xrep[layer + 1].ap()[0:NC_ * H0, :].opt()],
                        )

                if layer < 3:
                    nc.gpsimd.collective_compute(
                        "AllGather", mybir.AluOpType.bypass,
                        replica_groups=[list(range(NC_))],
                        ins=[xsh[layer + 1].ap()[H0:SLOTS, :].opt()],
                        outs=[xrep[layer + 1].ap()[NC_ * H0:, :].opt()],
                    )
    nc.compile()
    return nc


# --------------------------------------------------------------- host driver
def _pack_inputs(inputs, tables, cfg):
    import ml_dtypes
    BF16 = ml_dtypes.bfloat16
    WCOLS = 6 * P + 6

    feats = np.asarray(inputs["features"], np.float32)
    xsh0 = permute_rows(feats, tables["vert_at"], cfg).astype(BF16)
    H0 = min(160 * 128, cfg.SLOTS)
    xrep0 = np.concatenate([
        xsh0[:, 0:H0, :].reshape(cfg.NC * H0, P),
        xsh0[:, H0:, :].reshape(cfg.NC * (cfg.SLOTS - H0), P)])
    xrep0 = np.ascontiguousarray(xrep0)

    wts = np.zeros((P, WCOLS), np.float32)
    for l in range(3):
        wts[:, l * 256:l * 256 + P] = np.asarray(inputs[f"W1_{l}"], np.float32)
        wts[:, l * 256 + P:l * 256 + 2 * P] = np.asarray(inputs[f"W0_{l}"],
                                                         np.float32)
    wts[:, 768:771] = np.asarray(inputs["W1_3"], np.float32)
    wts[:, 771:774] = np.asarray(inputs["W0_3"], np.float32)
    wts = wts.astype(BF16)

    iot = np.tile(np.arange(256, dtype=np.float32)[None, :],
                  (P, 1)).astype(BF16)

    in_maps = []
    for c in range(cfg.NC):
        in_maps.append({
            "xrep0": xrep0,
            "x0sh": np.ascontiguousarray(xsh0[c]),
            "idx16": np.ascontiguousarray(tables["idx16"][c]),
            "rankt": np.ascontiguousarray(tables["rank"][c]).astype(BF16),
            "rankt2": np.ascontiguousarray(tables["rank2"][c]).astype(BF16),
            "wts": wts,
            "iot": iot,
        })
    return in_maps


def _np_fallback(inputs):
    x = np.asarray(inputs["features"], np.float32)
    e = np.asarray(inputs["edges"], np.int64)
    src, dst = e[:, 0], e[:, 1]

    def gc(x, i):
        h0 = x @ np.asarray(inputs[f"W0_{i}"], np.float32) + \
            np.asarray(inputs[f"b0_{i}"], np.float32)
        h1 = x @ np.asarray(inputs[f"W1_{i}"], np.float32) + \
            np.asarray(inputs[f"b1_{i}"], np.float32)
        agg = np.zeros_like(h0)
        np.add.at(agg, src, h1[dst])
        np.add.at(agg, dst, h1[src])
        return h0 + agg

    h = np.maximum(gc(x, 0), 0)
    h = np.maximum(gc(h, 1), 0)
    h = np.maximum(gc(h, 2), 0)
    h = h + x
    return gc(h, 3).reshape(2, 4, 40000, 3)


_NC_CACHE = {}
LAST_EXEC_TIME_NS = None


def kernel(**inputs):
    global LAST_EXEC_TIME_NS
    cfg = CFG_FULL
    feats = np.asarray(inputs["features"])
    assert feats.shape == (cfg.N, 128)

    for i in range(4):
        for b in ("b0", "b1"):
            v = inputs.get(f"{b}_{i}")
            if v is not None and np.any(np.asarray(v) != 0):
                return _np_fallback(inputs)

    from concourse.bass_utils import run_bass_kernel_spmd

    edges = np.asarray(inputs["edges"]).astype(np.int64)
    ek = hash(edges.tobytes())
    if _NC_CACHE.get("key") != ek:
        tables = build_tables(edges, cfg)
        _NC_CACHE.update(key=ek, tables=tables,
                         nc=build_nc(cfg, tables))
    nc = _NC_CACHE["nc"]
    tables = _NC_CACHE["tables"]
    in_maps = _pack_inputs(inputs, tables, cfg)

    import os
    trace = bool(int(os.environ.get("GNN_TRACE", "0")))
    res = run_bass_kernel_spmd(nc, in_maps, core_ids=list(range(cfg.NC)),
                               trace=trace)
    LAST_EXEC_TIME_NS = res.exec_time_ns

    vert_at = tables["vert_at"]
    full = np.zeros((cfg.N, 3), np.float32)
    for c in range(cfg.NC):
        o = np.asarray(res.results[c]["out"], np.float32)
        m = vert_at[c] >= 0
        full[vert_at[c][m]] = o[m]
    return full.reshape(2, 4, 40000, 3)



# revision 7
# speedup vs baseline: 1.0251x; 1.0251x over previous
"""MeshRefineNet GNN on 8 TRN2 cores — v3: prepared gathers + windowed AG.

v2 staged neighbor rows with dma_gather (gen_mode=0): the Pool/Q7 engine was
busy 10.0ms of the 11.2ms kernel, but over half of that was *embedded waits*
(the gather instruction carries its stg/idx/xrep data deps, so Q7 idles
inside the slice while DMAs land).  v3 decouples descriptor generation from
data movement:

  * every gather is issued with prepare_only=True on one of two SWDGE
    queues; a trigger_dma(count=None) fires it one run later.  Preps only
    depend on the (layer-invariant) index tables, so Q7 streams descriptor
    generation back-to-back; data deps (xrep windows, stg buffer reuse)
    ride on the cheap trigger instruction.
  * the replica layout is window-contiguous (row w*32768 + c*4096 + i) and
    the per-layer AllGather is split into 10 per-window collectives fired
    after every odd batch, so window w of layer l+1 is ready long before
    the first trigger that reads it.

Transforms, rank-table S build, and the per-group PSUM aggregation are
unchanged from v2.
"""
import sys
from collections import deque

if "/opt/trn_rl_repo" not in sys.path:
    sys.path.insert(0, "/opt/trn_rl_repo")

import numpy as np

P = 128
SUBW = 32
GB = 16            # groups per batch
WIN = 32768        # dma_gather int16 window (rows)
CH = 4096          # slots per (core, window): WIN = 8 * CH
MAX_RUN = 2048     # split longer runs (descriptor-ring safety)
NQ = 2             # SWDGE queues for prepared gathers


class Cfg:
    def __init__(self, N, E, NC=8, GROUPS=313):
        self.N, self.E, self.NC, self.GROUPS = N, E, NC, GROUPS
        self.SLOTS = GROUPS * 128
        self.NREP = NC * self.SLOTS
        self.NWIN = -(-self.NREP // WIN)
        self.NB = -(-GROUPS // GB)
        assert self.NREP >= N
        assert WIN == NC * CH


CFG_FULL = Cfg(N=320000, E=960000, NC=8, GROUPS=313)


def _slot_to_rep(slot, core, cfg):
    """Window-contiguous replica row for (core, slot)."""
    w = slot // CH
    nfull = cfg.SLOTS // CH
    ch_w = np.where(w < nfull, CH, cfg.SLOTS - nfull * CH)
    return w * WIN + core * ch_w + (slot - w * CH)


# ----------------------------------------------------------------- host prep
def build_tables(edges, cfg):
    N, NC, GROUPS = cfg.N, cfg.NC, cfg.GROUPS
    SLOTS, NB, NWIN = cfg.SLOTS, cfg.NB, cfg.NWIN

    src0 = edges[:, 0].astype(np.int64)
    dst0 = edges[:, 1].astype(np.int64)
    tgt = np.concatenate([src0, dst0])
    src = np.concatenate([dst0, src0])
    deg = np.bincount(tgt, minlength=N)

    # balanced vertex->slot assignment (degree snake over 32-row bins)
    NBINS = NC * GROUPS * 4
    n_items = NBINS * SUBW
    deg_ext = np.concatenate([deg, np.zeros(n_items - N, np.int64)])
    order = np.argsort(-deg_ext, kind="stable")
    grid = order.reshape(SUBW, NBINS).copy()
    for r in range(1, SUBW, 2):
        grid[r] = grid[r, ::-1]
    item_ids = grid.ravel()
    rr, bb = np.divmod(np.arange(SUBW * NBINS), NBINS)
    core_of = np.empty(n_items, np.int64)
    slot_of = np.empty(n_items, np.int64)
    core_of[item_ids] = bb // (GROUPS * 4)
    g_item = (bb % (GROUPS * 4)) // 4
    slot_of[item_ids] = g_item * 128 + (bb % 4) * SUBW + rr
    R = _slot_to_rep(slot_of, core_of, cfg)

    vert_at = np.full((NC, SLOTS), -1, np.int64)
    vert_at[core_of[:N], slot_of[:N]] = np.arange(N)

    c_t = core_of[tgt]
    s_t = slot_of[tgt]
    g_t = s_t // 128
    b_t = g_t // GB
    Rs = R[src]
    w_t = Rs // WIN

    # per (core, batch, window, group) counts -> uniform max lengths
    key = ((c_t * NB + b_t) * NWIN + w_t) * GROUPS + g_t
    cnt = np.bincount(key, minlength=NC * NB * NWIN * GROUPS) \
        .reshape(NC, NB, NWIN, GROUPS)
    Lmax = cnt.max(axis=0)          # [NB, NWIN, GROUPS]

    # ---- uniform walk: tiles / runs / mm schedule (core-independent) ----
    structure = []
    batchoff = []
    total = 0
    segstart = {}
    blockmap = {}                   # (tile_global, g) -> block index
    for b in range(NB):
        glo = b * GB
        ghi = min((b + 1) * GB, GROUPS)
        pos = 0
        runs = []
        tile_groups = []            # batch-local tile -> [g0, g1?]
        mms = []                    # (tile_local, block, g)
        for w in range(NWIN):
            run_start = pos
            for g in range(glo, ghi):
                L = int(Lmax[b, w, g])
                if L == 0:
                    continue
                t = pos // 128
                if pos % 128 != 0:
                    tg = tile_groups[t]
                    if g not in tg and len(tg) == 3:
                        pos = (t + 1) * 128
                segstart[(b, w, g)] = pos
                while L > 0:
                    t = pos // 128
                    while t >= len(tile_groups):
                        tile_groups.append([])
                    tg = tile_groups[t]
                    if g not in tg:
                        assert len(tg) < 3
                        tg.append(g)
                        mms.append((t, tg.index(g), g))
                        blockmap[(total // 128 + t, g)] = tg.index(g)
                    room = (t + 1) * 128 - pos
                    step = min(room, L)
                    pos += step
                    L -= step
            if pos % 128:
                pos = (pos // 128 + 1) * 128
            if pos > run_start:
                runs.append((w, run_start, pos - run_start))
        T_b = pos // 128
        # group-major MM lists (one PSUM bank session per group)
        gmms = {g: [] for g in range(glo, ghi)}
        for (t, blk, g) in mms:
            gmms[g].append((t, blk))
        for g in range(glo, ghi):
            assert gmms[g], f"group {g} empty in batch {b}"
        # compacted S2 slots for tiles holding a 3rd group
        s2loc = {}
        for t, tg in enumerate(tile_groups):
            if len(tg) == 3:
                s2loc[t] = len(s2loc)
        structure.append(dict(T=T_b, runs=runs, gmms=gmms,
                              glo=glo, ng=ghi - glo, s2loc=s2loc,
                              s2off=sum(len(st["s2loc"])
                                        for st in structure)))
        batchoff.append(total)
        total += T_b * 128

    T_total = total // 128
    T2_total = sum(len(st["s2loc"]) for st in structure)

    # ---- per-core tables: staged position of every half-edge ----
    # order within a segment: by (slot, source row)
    seg_of = ((c_t * NB + b_t) * NWIN + w_t) * GROUPS + g_t
    eorder = np.lexsort((Rs, s_t, seg_of))
    so = seg_of[eorder]
    first_in_seg = np.searchsorted(so, np.arange(NC * NB * NWIN * GROUPS))
    cum = np.arange(so.size) - first_in_seg[so]

    segstart_arr = np.zeros((NB, NWIN, GROUPS), np.int64)
    for (b, w, g), v in segstart.items():
        segstart_arr[b, w, g] = v
    boff_arr = np.asarray(batchoff, np.int64)

    et = eorder
    pos_local = segstart_arr[b_t[et], w_t[et], g_t[et]] + cum
    pos_global = boff_arr[b_t[et]] + pos_local
    tile_g = pos_global // 128
    part_g = pos_global % 128

    blk_of = np.zeros(et.size, np.int64)
    bm_keys = np.array([k[0] * GROUPS + k[1] for k in blockmap], np.int64)
    bm_vals = np.array(list(blockmap.values()), np.int64)
    bm = np.zeros(T_total * GROUPS, np.int64)
    bm[bm_keys] = bm_vals
    blk_of = bm[tile_g * GROUPS + g_t[et]]

    idx_lin = np.zeros((NC, total), np.int16)
    rank_lin = np.full((NC, total), 512.0, np.float32)
    rank2_lin = np.full((NC, total), 512.0, np.float32)
    cc = c_t[et]
    idx_lin[cc, pos_global] = (Rs[et] % WIN).astype(np.int16)
    r1v = np.where(blk_of < 2, s_t[et] % 128 + 128 * blk_of, 512)
    r2v = np.where(blk_of == 2, s_t[et] % 128, 512)
    rank_lin[cc, pos_global] = r1v.astype(np.float32)
    rank2_lin[cc, pos_global] = r2v.astype(np.float32)

    # wrapped idx layout per run: local i -> (partition i%16 (+16k), col i//16)
    idx16 = np.zeros((NC, 128, total // 16), np.int16)
    for b in range(NB):
        for (w, start, L) in structure[b]["runs"]:
            s0 = batchoff[b] + start
            blkv = idx_lin[:, s0:s0 + L].reshape(NC, L // 16, 16) \
                .transpose(0, 2, 1)                       # [NC, 16, L/16]
            idx16[:, :, s0 // 16:(s0 + L) // 16] = np.tile(blkv, (1, 8, 1))

    rank_tbl = rank_lin.reshape(NC, T_total, 128).transpose(0, 2, 1).copy()
    rank2_full = rank2_lin.reshape(NC, T_total, 128).transpose(0, 2, 1)
    rank2_tbl = np.full((NC, 128, max(T2_total, 1)), 512.0, np.float32)
    for b, st in enumerate(structure):
        bt0 = batchoff[b] // 128
        for t, p2 in st["s2loc"].items():
            rank2_tbl[:, :, st["s2off"] + p2] = rank2_full[:, :, bt0 + t]

    return dict(idx16=idx16, rank=rank_tbl, rank2=rank2_tbl,
                vert_at=vert_at,
                structure=structure, batchoff=batchoff, T_total=T_total,
                T2_total=T2_total,
                T_max=max(s["T"] for s in structure),
                T2_max=max(len(s["s2loc"]) for s in structure))


def permute_rows(x, vert_at, cfg):
    out = np.zeros((cfg.NC, cfg.SLOTS, x.shape[1]), x.dtype)
    m = vert_at >= 0
    out[m] = x[vert_at[m]]
    return out


# ------------------------------------------------------------- device build
def build_nc(cfg, tables):
    import concourse.bacc as bacc
    import concourse.tile as tile
    import concourse.mybir as mybir
    from concourse import library_config

    BF = mybir.dt.bfloat16
    F32 = mybir.dt.float32
    I16 = mybir.dt.int16
    RELU = mybir.ActivationFunctionType.Relu
    WCOLS = 6 * P + 6

    NC_, GROUPS, SLOTS, NREP = cfg.NC, cfg.GROUPS, cfg.SLOTS, cfg.NREP
    structure = tables["structure"]
    batchoff = tables["batchoff"]
    T_total = tables["T_total"]
    T_max = tables["T_max"]
    HALF = -(-T_max // 2)
    NFULL = SLOTS // CH

    nc = bacc.Bacc(None, target_bir_lowering=False, debug=False,
                   num_swdge_queues=NQ)
    xrep0 = nc.declare_dram_parameter("xrep0", [NREP, P], BF, isOutput=False)
    x0sh = nc.declare_dram_parameter("x0sh", [SLOTS, P], BF, isOutput=False)
    idx16 = nc.declare_dram_parameter("idx16", [P, T_total * 8], I16,
                                      isOutput=False)
    rankt = nc.declare_dram_parameter("rankt", [P, T_total], BF,
                                      isOutput=False)
    T2_total = max(tables["T2_total"], 1)
    T2_max = max(tables["T2_max"], 1)
    rankt2 = nc.declare_dram_parameter("rankt2", [P, T2_total], BF,
                                       isOutput=False)
    wts = nc.declare_dram_parameter("wts", [P, WCOLS], BF, isOutput=False)
    iot = nc.declare_dram_parameter("iot", [P, 256], BF, isOutput=False)
    out = nc.declare_dram_parameter("out", [SLOTS, 3], F32, isOutput=True)

    xsh = {l: nc.dram_tensor(f"xsh{l}", [SLOTS, P], BF) for l in (1, 2, 3)}
    xrep = {l: nc.dram_tensor(f"xrep{l}", [NREP, P], BF, addr_space="Shared")
            for l in (1, 2, 3)}

    with tile.TileContext(nc) as tc:
        dsem = [nc.alloc_semaphore(f"gsem{q}") for q in range(NQ)]
        with (
            tc.tile_pool(name="res", bufs=1) as res,
            tc.tile_pool(name="idx", bufs=2) as idx_p,
            tc.tile_pool(name="stg", bufs=2) as stg_p,
            tc.tile_pool(name="s", bufs=2) as s_p,
            tc.tile_pool(name="xt", bufs=2) as xt_p,
            tc.tile_pool(name="x0t", bufs=2) as x0t_p,
            tc.tile_pool(name="xat", bufs=4) as xat_p,
            tc.tile_pool(name="ob", bufs=2) as ob_p,
            tc.tile_pool(name="psA", bufs=6, space="PSUM") as psA,
            tc.tile_pool(name="psB", bufs=2, space="PSUM") as psB,
        ):
            nc.gpsimd.load_library(library_config.attnmlp)
            rank_sb = res.tile([P, T_total], BF)
            nc.sync.dma_start(out=rank_sb[:], in_=rankt.ap())
            rank2_sb = res.tile([P, T2_total], BF)
            nc.sync.dma_start(out=rank2_sb[:], in_=rankt2.ap())
            iota_sb = res.tile([P, HALF * 256], BF)
            nc.sync.dma_start(
                out=iota_sb[:].rearrange("p (t c) -> p t c", c=256),
                in_=iot.ap().rearrange("p (o c) -> p o c", o=1)
                    .to_broadcast([P, HALF, 256]))
            wts_sb = res.tile([P, WCOLS], BF)
            nc.sync.dma_start(out=wts_sb[:], in_=wts.ap())

            # ---- prepared-gather bookkeeping ----
            run_q = deque()     # queue numbers of untriggered preps
            run_ctr = [0]

            import os as _os
            USE_PREP = bool(int(_os.environ.get("GNN_PREP", "1")))

            def prep_gather(stg, lo_t, hi_t, rep_lo, rep_hi, idxb, i_lo, i_hi,
                            Lc, rep_ap):
                if not USE_PREP:
                    nc.gpsimd.dma_gather(
                        out_ap=stg[:, lo_t:hi_t, :],
                        in_ap=rep_ap[rep_lo:rep_hi, :],
                        idxs_ap=idxb[:, i_lo:i_hi],
                        num_idxs=Lc,
                        num_idxs_reg=Lc,
                        elem_size=P,
                        single_packet=False,
                    )
                    return
                q = run_ctr[0] % NQ
                run_ctr[0] += 1
                nc.gpsimd.dma_gather(
                    out_ap=stg[:, lo_t:hi_t, :],
                    in_ap=rep_ap[rep_lo:rep_hi, :],
                    idxs_ap=idxb[:, i_lo:i_hi],
                    num_idxs=Lc,
                    num_idxs_reg=Lc,
                    elem_size=P,
                    single_packet=False,
                    prepare_only=True,
                    sem=dsem[q],
                    queue_num=q,
                )
                run_q.append(q)
                if len(run_q) > 1:
                    nc.gpsimd.trigger_dma(count=None,
                                          queue_num=run_q.popleft())

            def flush_gathers():
                while run_q:
                    nc.gpsimd.trigger_dma(count=None,
                                          queue_num=run_q.popleft())

            def emit_ag(layer, w_ag):
                lo = w_ag * CH
                hi = min(lo + CH, SLOTS)
                nc.gpsimd.collective_compute(
                    "AllGather", mybir.AluOpType.bypass,
                    replica_groups=[list(range(NC_))],
                    ins=[xsh[layer + 1].ap()[lo:hi, :].opt()],
                    outs=[xrep[layer + 1].ap()
                          [w_ag * WIN:w_ag * WIN + (hi - lo) * NC_, :]
                          .opt()],
                )

            for layer in range(4):
                rep_ap = xrep0.ap() if layer == 0 else xrep[layer].ap()
                sh_ap = x0sh.ap() if layer == 0 else xsh[layer].ap()
                if layer < 3:
                    w1 = wts_sb[:, layer * 256: layer * 256 + P]
                    w0 = wts_sb[:, layer * 256 + P: layer * 256 + 2 * P]
                else:
                    w1 = wts_sb[:, 768:771]
                    w0 = wts_sb[:, 771:774]

                idxb_tiles = {}

                def load_idx(b):
                    st_b = structure[b]
                    boff_b = batchoff[b]
                    t_ = idx_p.tile([P, T_max * 8], I16, tag="idx")
                    nc.sync.dma_start(
                        out=t_[:, 0:st_b["T"] * 8],
                        in_=idx16.ap()[:, boff_b // 16:
                                       (boff_b + st_b["T"] * 128) // 16])
                    idxb_tiles[b] = t_

                load_idx(0)
                for b, st in enumerate(structure):
                    T_b, ng, glo = st["T"], st["ng"], st["glo"]
                    boff = batchoff[b]
                    bt0 = boff // 128

                    idxb = idxb_tiles.pop(b)
                    stg = stg_p.tile([P, T_max, P], BF, tag="stg")
                    for (w, start, L) in st["runs"]:
                        w_hi = min((w + 1) * WIN, NREP)
                        o = 0
                        while o < L:
                            Lc = min(MAX_RUN, L - o)
                            prep_gather(
                                stg,
                                (start + o) // 128, (start + o + Lc) // 128,
                                w * WIN, w_hi,
                                idxb,
                                (start + o) // 16, (start + o + Lc) // 16,
                                Lc, rep_ap)
                            o += Lc
                    # fire remaining triggers before anything downstream
                    # (the batch-end AllGather transitively waits on these
                    # DMAs; an untriggered prep behind it would deadlock)
                    flush_gathers()
                    if b + 1 < len(structure):
                        load_idx(b + 1)

                    s_tiles = []
                    for hb in range(2):
                        t0 = hb * HALF
                        t1 = min(T_b, (hb + 1) * HALF)
                        if t0 >= t1:
                            s_tiles.append(None)
                            continue
                        S = s_p.tile([P, HALF * 256], BF, tag="s")
                        nc.vector.tensor_tensor(
                            out=S[:, 0:(t1 - t0) * 256]
                                .rearrange("p (t c) -> p t c", c=256),
                            in0=rank_sb[:, bt0 + t0:bt0 + t1]
                                .rearrange("p (t o) -> p t o", o=1)
                                .to_broadcast([P, t1 - t0, 256]),
                            in1=iota_sb[:, 0:(t1 - t0) * 256]
                                .rearrange("p (t c) -> p t c", c=256),
                            op=mybir.AluOpType.is_equal,
                        )
                        s_tiles.append((S, t0))

                    T2_b = len(st["s2loc"])
                    s2l = st["s2loc"]
                    if T2_b:
                        S2b = s_p.tile([P, T2_max * P], BF, tag="s2")
                        nc.vector.tensor_tensor(
                            out=S2b[:, 0:T2_b * P]
                                .rearrange("p (t c) -> p t c", c=P),
                            in0=rank2_sb[:, st["s2off"]:st["s2off"] + T2_b]
                                .rearrange("p (t o) -> p t o", o=1)
                                .to_broadcast([P, T2_b, P]),
                            in1=iota_sb[:, 0:T2_b * 256]
                                .rearrange("p (t c) -> p t c", c=256)
                                [:, :, 0:P],
                            op=mybir.AluOpType.is_equal,
                        )

                    # ---- per-group aggregation + transforms ----
                    g0 = glo
                    xt = xt_p.tile([P, GB * P], BF, tag="xt")
                    nc.sync.dma_start(out=xt[:, 0:ng * P],
                                      in_=sh_ap[g0 * P:(g0 + ng) * P, :],
                                      transpose=True)
                    if layer == 2:
                        x0v = x0t_p.tile([P, GB, P], BF, tag="x0v")
                        nc.sync.dma_start(
                            out=x0v[:, 0:ng, :],
                            in_=x0sh.ap()[g0 * P:(g0 + ng) * P, :]
                                .rearrange("(g p) c -> p g c", p=P))
                    if layer < 3:
                        obuf = ob_p.tile([P, GB, P], BF, tag="ob")
                    else:
                        obuf = ob_p.tile([P, GB, 3], F32, tag="ob3")

                    for gl in range(ng):
                        glist = st["gmms"][glo + gl]
                        pg = psA.tile([P, P], F32, tag="pg")
                        for i, (t, blk) in enumerate(glist):
                            if blk < 2:
                                S, t0 = s_tiles[0] if t < HALF else s_tiles[1]
                                rhs = S[:, (t - t0) * 256 + blk * 128:
                                        (t - t0) * 256 + blk * 128 + 128]
                            else:
                                rhs = S2b[:, s2l[t] * P:(s2l[t] + 1) * P]
                            nc.tensor.matmul(
                                pg[:],
                                lhsT=stg[:, t, :],
                                rhs=rhs,
                                start=(i == 0), stop=(i == len(glist) - 1),
                            )
                        xat = xat_p.tile([P, P], BF)
                        nc.vector.tensor_copy(out=xat[:], in_=pg[:])
                        if layer < 3:
                            ps2 = psB.tile([P, P], F32, tag="ps2")
                            nc.tensor.matmul(ps2[:], lhsT=xat[:], rhs=w1,
                                             start=True, stop=False)
                            nc.tensor.matmul(ps2[:],
                                             lhsT=xt[:, gl * P:(gl + 1) * P],
                                             rhs=w0, start=False, stop=True)
                            if layer == 2:
                                rl = xat_p.tile([P, P], BF, tag="rl")
                                nc.scalar.activation(out=rl[:], in_=ps2[:],
                                                     func=RELU)
                                nc.vector.tensor_add(out=obuf[:, gl, :],
                                                     in0=rl[:],
                                                     in1=x0v[:, gl, :])
                            else:
                                nc.scalar.activation(out=obuf[:, gl, :],
                                                     in_=ps2[:], func=RELU)
                        else:
                            ps2 = psB.tile([P, P], F32, tag="ps2")
                            nc.tensor.matmul(ps2[:, 0:3], lhsT=xat[:], rhs=w1,
                                             start=True, stop=False)
                            nc.tensor.matmul(ps2[:, 0:3],
                                             lhsT=xt[:, gl * P:(gl + 1) * P],
                                             rhs=w0, start=False, stop=True)
                            nc.vector.tensor_copy(out=obuf[:, gl, :],
                                                  in_=ps2[:, 0:3])

                    if layer < 3:
                        dst = xsh[layer + 1].ap()[g0 * P:(g0 + ng) * P, :] \
                            .rearrange("(g p) c -> p g c", p=P)
                    else:
                        dst = out.ap()[g0 * P:(g0 + ng) * P, :] \
                            .rearrange("(g p) c -> p g c", p=P)
                    nc.sync.dma_start(out=dst, in_=obuf[:, 0:ng, :])

                    # window-contiguous AllGather: window w is complete once
                    # batch 2w+1 has stored its slots; the doorbell is
                    # emitted one batch later so its store-wait is already
                    # satisfied when the Pool sequencer reaches it.
                    if layer < 3 and b >= 2 and b % 2 == 0:
                        emit_ag(layer, (b - 2) // 2)

                flush_gathers()
                if layer < 3:
                    for w_ag in range((len(structure) - 2) // 2, cfg.NWIN):
                        emit_ag(layer, w_ag)
    nc.compile()
    return nc


# --------------------------------------------------------------- host driver
def _pack_inputs(inputs, tables, cfg):
    import ml_dtypes
    BF16 = ml_dtypes.bfloat16
    WCOLS = 6 * P + 6

    feats = np.asarray(inputs["features"], np.float32)
    xsh0 = permute_rows(feats, tables["vert_at"], cfg).astype(BF16)
    slots = np.arange(cfg.SLOTS)
    xrep0 = np.zeros((cfg.NREP, P), BF16)
    for c in range(cfg.NC):
        xrep0[_slot_to_rep(slots, c, cfg)] = xsh0[c]

    wts = np.zeros((P, WCOLS), np.float32)
    for l in range(3):
        wts[:, l * 256:l * 256 + P] = np.asarray(inputs[f"W1_{l}"], np.float32)
        wts[:, l * 256 + P:l * 256 + 2 * P] = np.asarray(inputs[f"W0_{l}"],
                                                         np.float32)
    wts[:, 768:771] = np.asarray(inputs["W1_3"], np.float32)
    wts[:, 771:774] = np.asarray(inputs["W0_3"], np.float32)
    wts = wts.astype(BF16)

    iot = np.tile(np.arange(256, dtype=np.float32)[None, :],
                  (P, 1)).astype(BF16)

    in_maps = []
    for c in range(cfg.NC):
        in_maps.append({
            "xrep0": xrep0,
            "x0sh": np.ascontiguousarray(xsh0[c]),
            "idx16": np.ascontiguousarray(tables["idx16"][c]),
            "rankt": np.ascontiguousarray(tables["rank"][c]).astype(BF16),
            "rankt2": np.ascontiguousarray(tables["rank2"][c]).astype(BF16),
            "wts": wts,
            "iot": iot,
        })
    return in_maps


def _np_fallback(inputs):
    x = np.asarray(inputs["features"], np.float32)
    e = np.asarray(inputs["edges"], np.int64)
    src, dst = e[:, 0], e[:, 1]

    def gc(x, i):
        h0 = x @ np.asarray(inputs[f"W0_{i}"], np.float32) + \
            np.asarray(inputs[f"b0_{i}"], np.float32)
        h1 = x @ np.asarray(inputs[f"W1_{i}"], np.float32) + \
            np.asarray(inputs[f"b1_{i}"], np.float32)
        agg = np.zeros_like(h0)
        np.add.at(agg, src, h1[dst])
        np.add.at(agg, dst, h1[src])
        return h0 + agg

    h = np.maximum(gc(x, 0), 0)
    h = np.maximum(gc(h, 1), 0)
    h = np.maximum(gc(h, 2), 0)
    h = h + x
    return gc(h, 3).reshape(2, 4, 40000, 3)


_NC_CACHE = {}
LAST_EXEC_TIME_NS = None


def kernel(**inputs):
    global LAST_EXEC_TIME_NS
    cfg = CFG_FULL
    feats = np.asarray(inputs["features"])
    assert feats.shape == (cfg.N, 128)

    for i in range(4):
        for b in ("b0", "b1"):
            v = inputs.get(f"{b}_{i}")
            if v is not None and np.any(np.asarray(v) != 0):
                return _np_fallback(inputs)

    from concourse.bass_utils import run_bass_kernel_spmd

    edges = np.asarray(inputs["edges"]).astype(np.int64)
    ek = hash(edges.tobytes())
    if _NC_CACHE.get("key") != ek:
        tables = build_tables(edges, cfg)
        _NC_CACHE.update(key=ek, tables=tables,
                         nc=build_nc(cfg, tables))
    nc = _NC_CACHE["nc"]
    tables = _NC_CACHE["tables"]
    in_maps = _pack_inputs(inputs, tables, cfg)

    import os
    trace = bool(int(os.environ.get("GNN_TRACE", "0")))
    res = run_bass_kernel_spmd(nc, in_maps, core_ids=list(range(cfg.NC)),
                               trace=trace)
    LAST_EXEC_TIME_NS = res.exec_time_ns

    vert_at = tables["vert_at"]
    full = np.zeros((cfg.N, 3), np.float32)
    for c in range(cfg.NC):
        o = np.asarray(res.results[c]["out"], np.float32)
        m = vert_at[c] >= 0
        full[vert_at[c][m]] = o[m]
    return full.reshape(2, 4, 40000, 3)
